# revision 63
# baseline (speedup 1.0000x reference)
"""Causal self-attention on 8 trn2 NeuronCores.

Sharding: tensor-parallel over heads (2 heads/core) for QKV+attention, then an
8-rank AllToAll reshards from head-split to row-split for the output
projection; each core computes 512 rows of the final output.

All matmuls run in bf16 with f32 PSUM accumulation.  Attention is computed in
"scores transposed" layout ([keys, queries] on chip) so no on-chip transposes
of the probability matrix are needed; softmax denominators come from a ones
column appended to V, and the causal mask is a multiplicative {0,1} bf16 mask
applied after exp (safe: scores are O(6), no overflow without max-subtraction).
"""

import numpy as np
import ml_dtypes

B, T, D, H, HD = 2, 2048, 1024, 16, 64
NCORES = 8
R = B * T              # 4096 global rows (b*T + t)
HPC = H // NCORES      # 2 heads per core
HDIM = HPC * HD        # 128 dims per core
ROWS_PER_CORE = R // NCORES  # 512
NKT = D // 128         # 8 contraction tiles
NSTRIP = T // 512      # 4 query strips per batch
NKB = T // 128         # 16 key blocks per batch

_BF16 = ml_dtypes.bfloat16
_cache = {}


def _patch_tile_drain():
    """This walrus build rejects >1 sync wait on SP CTRL instructions; split
    the Tile tail-drain waits across single-wait nops."""
    import concourse.mybir as mybir
    import concourse.tile as tile_mod
    from concourse.vector_clock import ScopedClock

    if getattr(tile_mod.TileContext, "_drain_patched", False):
        return

    def _drain_and_barrier(self, tick_clock, wait_clock):
        nc = self.nc
        dummy = mybir.InstNoOp(
            name=nc.get_next_instruction_name(),
            engine=mybir.EngineType.SP,
            ins=[],
            outs=[],
        )
        wait_clock.add_sem_waits(dummy, ScopedClock({None: tick_clock.global_clock}))
        waits = list(dummy.sync_info.on_wait) if dummy.sync_info else []
        for i in range(len(waits)):
            w = nc.sync.nop(nofuse=True, hint="tail_drain_wait")
            w.ins.sync_info = mybir.SyncInfo(on_wait=waits[i : i + 1], on_update=[])
        nc.sync.drain()
        nc.all_engine_barrier()
        assert self.sems is not None
        popped = nc._tile_sem_poison_stack.pop()
        assert popped is self._sem_poison
        nc.clear_and_free_semaphores(list(self.sems.allocated().values()))
        nc.all_engine_barrier()

    tile_mod.TileContext._drain_and_barrier = _drain_and_barrier

    # Body instructions can also accumulate >2 waits (CTRL structs take 1,
    # other structs 2 on this walrus).  Before lowering, move excess waits
    # onto single-wait nops inserted just before the instruction on the same
    # engine stream.
    _orig_lower = tile_mod.TileContext._lower_ordered_insts

    def _lower_split_waits(self, ordered):
        nc = self.nc
        for bb_name, insts in ordered.items():
            new_insts = []
            for inst in insts:
                si = getattr(inst, "sync_info", None)
                waits = list(si.on_wait) if si is not None and si.on_wait else []
                limit = 1
                if len(waits) > limit and inst.engine is not None:
                    keep = waits[: limit - 1] if limit > 1 else []
                    spill = waits[len(keep) :][:-1]
                    keep = keep + [waits[-1]]
                    for w in spill:
                        nop = mybir.InstNoOp(
                            name=nc.get_next_instruction_name(),
                            engine=inst.engine,
                            ins=[],
                            outs=[],
                        )
                        nop.sync_info = mybir.SyncInfo(on_wait=[w], on_update=[])
                        nop.debug = inst.debug
                        new_insts.append(nop)
                    inst.sync_info = mybir.SyncInfo(
                        on_wait=keep, on_update=list(si.on_update or [])
                    )
                new_insts.append(inst)
            ordered[bb_name] = new_insts
        return _orig_lower(self, ordered)

    tile_mod.TileContext._lower_ordered_insts = _lower_split_waits
    tile_mod.TileContext._drain_patched = True


def _build():
    import concourse.bass as bass
    import concourse.mybir as mybir
    import concourse.tile as tile
    from concourse.tile import add_dep_helper
    from concourse.masks import make_identity

    _patch_tile_drain()
    f32 = mybir.dt.float32
    bf16 = mybir.dt.bfloat16

    nc = bass.Bass("TRN2", target_bir_lowering=False, debug=False, num_devices=NCORES)

    # ---- DRAM I/O (per core) ----
    xT = nc.dram_tensor("xT", [D, R], bf16, kind="ExternalInput").ap()
    wqT = nc.dram_tensor("wqT", [D, HDIM], bf16, kind="ExternalInput").ap()
    wkT = nc.dram_tensor("wkT", [D, HDIM], bf16, kind="ExternalInput").ap()
    wvT = nc.dram_tensor("wvT", [D, HDIM], bf16, kind="ExternalInput").ap()
    bq_s = nc.dram_tensor("bq_s", [HDIM, 1], f32, kind="ExternalInput").ap()
    bk_s = nc.dram_tensor("bk_s", [HDIM, 1], f32, kind="ExternalInput").ap()
    bv_s = nc.dram_tensor("bv_s", [HDIM, 1], f32, kind="ExternalInput").ap()
    woT = nc.dram_tensor("woT", [D, D], bf16, kind="ExternalInput").ap()
    bo_row = nc.dram_tensor("bo_row", [1, D], bf16, kind="ExternalInput").ap()
    masks_d = nc.dram_tensor("masks", [128, 4 * 1024], bf16, kind="ExternalInput").ap()
    out = nc.dram_tensor("out", [ROWS_PER_CORE, D], f32, kind="ExternalOutput").ap()

    # collective bounce buffers: shard j = rows [128j, 128(j+1))
    cc_in = nc.dram_tensor("cc_in", [NCORES * HDIM, 512], bf16).ap()
    cc_out = nc.dram_tensor("cc_out", [NCORES * HDIM, 512], bf16).ap()
    # per-strip softmax reciprocals: rows 2j (head A) and 2j+1 (head B)
    rec_d = nc.dram_tensor("rec_d", [2 * NCORES, 512], bf16).ap()


    with tile.TileContext(nc) as tc:
        import contextlib

        with contextlib.ExitStack() as ctx:
            singles = ctx.enter_context(tc.tile_pool(name="singles", bufs=1))

            # ---- constants / weights into SBUF (batched strided DMAs) ----
            # SP feeds the QKV-critical tensors (w, biases, even x tiles);
            # Pool feeds odd x tiles and the later-needed masks/wo/bo.
            w_sb = {}
            for name, src in (("q", wqT), ("k", wkT), ("v", wvT)):
                t = singles.tile([128, NKT, HDIM], bf16, tag=f"w{name}", name=f"w{name}")
                nc.sync.dma_start(
                    out=t, in_=src.rearrange("(k p) c -> p k c", p=128)
                )
                w_sb[name] = t
            bias_sb = {}
            for name, src in (("q", bq_s), ("k", bk_s), ("v", bv_s)):
                t = singles.tile([HDIM, 1], f32, tag=f"b{name}", name=f"b{name}")
                nc.sync.dma_start(out=t, in_=src)
                bias_sb[name] = t
            xt_sb = []
            feed_engs = [nc.sync, nc.gpsimd]
            for k in range(NKT):
                t = singles.tile([128, R], bf16, tag=f"xt{k}", name=f"xt{k}")
                feed_engs[k % 2].dma_start(out=t, in_=xT[128 * k : 128 * (k + 1), :])
                xt_sb.append(t)
            mask_sb = singles.tile([128, 4 * 1024], bf16, tag="mask")
            nc.gpsimd.dma_start(out=mask_sb, in_=masks_d)
            wo_sb = singles.tile([128, NKT, D], bf16, tag="wo")
            nc.gpsimd.dma_start(out=wo_sb, in_=woT.rearrange("(k p) c -> p k c", p=128))
            bo_sb = singles.tile([1, D], bf16, tag="bo")
            nc.gpsimd.dma_start(out=bo_sb, in_=bo_row)
            ident = singles.tile([128, 128], bf16, tag="ident")
            make_identity(nc, ident)
            ones_row = singles.tile([1, 128], bf16, tag="ones")
            nc.vector.memset(ones_row, 1.0)


            # persistent activations
            qT_sb = singles.tile([128, R], bf16, tag="qT")   # rows 0-63 head A dims
            kT_sb = singles.tile([128, R], bf16, tag="kT")
            vT_sb = singles.tile([128, R], bf16, tag="vT")
            # v in [keys, dims] layout per key block kb:
            #   cols 0:64 = vA, 64 = ones, 65:129 = vB, 129 = ones
            # so lhsT for head h = cols [65h, 65h+65) = [v_h | ones]:
            # attnout at psum rows 0-63, softmax sums at row 64.
            v_ab = singles.tile([128, R // 128, 130], bf16, tag="vab")

            # ---- phase 1: projections, V first so its transpose is off the
            # critical path; one 6-bank psum pool, k-outer over n-groups of 4.
            nc.vector.memset(v_ab[:, :, 64:65], 1.0)
            nc.vector.memset(v_ab[:, :, 129:130], 1.0)
            with tc.tile_pool(name="qkv_ps", bufs=8, space="PSUM") as pp:
                for name, dest in (("v", vT_sb), ("q", qT_sb), ("k", kT_sb)):
                    for g in range(2):
                        ps = []
                        for n in range(4):
                            ps.append(
                                pp.tile(
                                    [128, 512], f32, tag="qkvps",
                                    name=f"ps_{name}{g}{n}",
                                )
                            )
                        for k in range(NKT):
                            for n in range(4):
                                gn = 4 * g + n
                                nc.tensor.matmul(
                                    ps[n],
                                    w_sb[name][:, k, :],
                                    xt_sb[k][:, 512 * gn : 512 * (gn + 1)],
                                    start=(k == 0),
                                    stop=(k == NKT - 1),
                                )
                        for n in range(4):
                            gn = 4 * g + n
                            nc.vector.tensor_scalar_add(
                                dest[:, 512 * gn : 512 * (gn + 1)],
                                ps[n],
                                bias_sb[name],
                            )
                    if name == "v":
                        # transpose V to [keys, dims] using the same pool
                        for kb in range(R // 128):
                            ps = pp.tile([128, 1024], bf16, tag="qkvps")
                            tout = ps[:, 0:128]
                            nc.tensor.transpose(
                                tout, vT_sb[:, 128 * kb : 128 * (kb + 1)], ident
                            )
                            # strided copy: halves -> cols [0:64] and [65:129]
                            nc.vector.tensor_copy(
                                v_ab[:, kb, :].rearrange(
                                    "p (g c) -> p g c", c=65
                                )[:, :, 0:64],
                                tout.rearrange("p (g c) -> p g c", c=64),
                            )

            # ---- phase 3: attention per (batch, strip-pair) ----
            # strips are processed in pairs (0,1) and (2,3); for key blocks
            # visible to both strips the scores psum is [128, 1024] (cols
            # 0-511 strip s, 512-1023 strip s+1) and exp runs once over it.
            import concourse.bass as _bass

            cc_writes = []
            with (
                tc.tile_pool(name="sc_ps", bufs=2, space="PSUM") as scp,
                tc.tile_pool(name="av_ps", bufs=4, space="PSUM") as avp,
                tc.tile_pool(name="p_sb", bufs=10) as ppool,
                tc.tile_pool(name="att_sb", bufs=10) as apool,
                tc.tile_pool(name="bc_sb", bufs=8) as bpool,
                tc.tile_pool(name="rec_sb", bufs=4) as rpool,
            ):
                for b in range(B):
                    for s0 in (0, 2):
                        s1 = s0 + 1
                        j0, j1 = b * NSTRIP + s0, b * NSTRIP + s1
                        qc0 = slice(T * b + 512 * s0, T * b + 512 * (s0 + 1))
                        qc1 = slice(T * b + 512 * s1, T * b + 512 * (s1 + 1))
                        # attnV psum per (head, strip): rows 0=sumA.. see v_ab
                        psV = {}
                        for h in ("A", "B"):
                            for sx, jx in ((s0, j0), (s1, j1)):
                                psV[(h, sx)] = avp.tile(
                                    [128, 512], f32, tag="av", name=f"psV_{h}{jx}"
                                )
                        nkb0, nkb1 = 4 * (s0 + 1), 4 * (s1 + 1)
                        for kb in range(nkb1):
                            krange = slice(T * b + 128 * kb, T * b + 128 * (kb + 1))
                            gkb = (T // 128) * b + kb
                            both = kb < nkb0
                            p_of = {}
                            # pass 1: scores + exp + mask for BOTH heads, so
                            # PE fills with head-B scores while head-A exps
                            for hi, h in enumerate(("A", "B")):
                                rows = slice(64 * hi, 64 * (hi + 1))
                                psS = scp.tile([128, 1024], f32, tag="sc")
                                p = ppool.tile([128, 1024], bf16, tag="p")
                                p_of[h] = p
                                scale = 1.0 / float(np.sqrt(HD))
                                if both:
                                    # cols [0:off) of the s0 half are fully
                                    # causally masked -> skip them entirely
                                    m = kb - 4 * s0
                                    off = 128 * m if m >= 0 else 0
                                    nc.tensor.matmul(
                                        psS[:, off:512],
                                        kT_sb[rows, krange],
                                        qT_sb[
                                            rows,
                                            T * b + 512 * s0 + off
                                            : T * b + 512 * (s0 + 1),
                                        ],
                                        start=True,
                                        stop=True,
                                        tile_position=(64 * hi, 0),
                                    )
                                    nc.tensor.matmul(
                                        psS[:, 512:1024],
                                        kT_sb[rows, krange],
                                        qT_sb[rows, qc1],
                                        start=True,
                                        stop=True,
                                        tile_position=(64 * hi, 0),
                                    )
                                    nc.scalar.activation(
                                        out=p[:, off:1024],
                                        in_=psS[:, off:1024],
                                        func=mybir.ActivationFunctionType.Exp,
                                        scale=scale,
                                    )
                                    if m >= 0:
                                        # mask strip s0 half; s1 fully visible
                                        nc.vector.tensor_mul(
                                            p[:, off:1024],
                                            p[:, off:1024],
                                            mask_sb[
                                                :,
                                                1024 * m + off : 1024 * (m + 1),
                                            ],
                                        )
                                else:
                                    m = kb - 4 * s1
                                    off = 128 * m if m >= 0 else 0
                                    nc.tensor.matmul(
                                        psS[:, 512 + off : 1024],
                                        kT_sb[rows, krange],
                                        qT_sb[
                                            rows,
                                            T * b + 512 * s1 + off
                                            : T * b + 512 * (s1 + 1),
                                        ],
                                        start=True,
                                        stop=True,
                                        tile_position=(64 * hi, 0),
                                    )
                                    nc.scalar.activation(
                                        out=p[:, 512 + off : 1024],
                                        in_=psS[:, 512 + off : 1024],
                                        func=mybir.ActivationFunctionType.Exp,
                                        scale=scale,
                                    )
                                    if m >= 0:
                                        nc.vector.tensor_mul(
                                            p[:, 512 + off : 1024],
                                            p[:, 512 + off : 1024],
                                            mask_sb[
                                                :,
                                                1024 * m + off : 1024 * m + 512,
                                            ],
                                        )
                            # pass 2: attnV accumulate; lhsT = [v_h | ones]:
                            # attnout rows 0-63, softmax sums row 64.
                            # Sliced cols [0:off) were written by earlier
                            # (non-diagonal) key blocks of the strip.
                            m = kb - 4 * (s0 if both else s1)
                            off = 128 * m if m >= 0 else 0
                            for hi, h in enumerate(("A", "B")):
                                p = p_of[h]
                                lhsT = v_ab[:, gkb, 65 * hi : 65 * hi + 65]
                                if both:
                                    nc.tensor.matmul(
                                        psV[(h, s0)][0:65, off:512],
                                        lhsT,
                                        p[:, off:512],
                                        start=(kb == 0),
                                        stop=(kb == nkb0 - 1),
                                    )
                                    nc.tensor.matmul(
                                        psV[(h, s1)][0:65, 0:512],
                                        lhsT,
                                        p[:, 512:1024],
                                        start=(kb == 0),
                                        stop=(kb == nkb1 - 1),
                                    )
                                else:
                                    nc.tensor.matmul(
                                        psV[(h, s1)][0:65, off:512],
                                        lhsT,
                                        p[:, 512 + off : 1024],
                                        start=False,
                                        stop=(kb == nkb1 - 1),
                                    )
                        # normalization per strip of the pair (sums at row 64):
                        # reciprocal -> DRAM bounce -> partition-broadcast DMA
                        # -> multiply attnout rows (keeps compute engines free)
                        for sx, jx in ((s0, j0), (s1, j1)):
                            rec = rpool.tile([128, 1024], bf16, tag="rec")
                            with nc.allow_low_precision("bf16 softmax recip"):
                                nc.vector.reciprocal(
                                    rec[64:65, 0:512], psV[("A", sx)][64:65, :]
                                )
                                nc.vector.reciprocal(
                                    rec[64:65, 512:1024], psV[("B", sx)][64:65, :]
                                )
                            (nc.sync if sx % 2 == 0 else nc.gpsimd).dma_start(
                                out=rec_d[2 * jx : 2 * jx + 2, :], in_=rec[64:65, :]
                            )
                            for hi, h in enumerate(("A", "B")):
                                r_ap = rec_d[2 * jx + hi : 2 * jx + hi + 1, :]
                                bc_ap = _bass.AP(
                                    tensor=r_ap.tensor,
                                    offset=r_ap.offset,
                                    ap=[[0, 64]] + list(r_ap.ap[1:]),
                                )
                                bc = bpool.tile([64, 512], bf16, tag="bc")
                                (nc.sync if hi == 0 else nc.gpsimd).dma_start(
                                    out=bc, in_=bc_ap
                                )
                                att = apool.tile([64, 512], bf16, tag="att")
                                nc.vector.tensor_mul(
                                    att, psV[(h, sx)][0:64, :], bc
                                )
                                eng = nc.sync if hi == 0 else nc.gpsimd
                                wr = eng.dma_start(
                                    out=cc_in[
                                        128 * jx + 64 * hi : 128 * jx + 64 * (hi + 1),
                                        :,
                                    ],
                                    in_=att,
                                )
                                cc_writes.append(wr)

            # ---- phase 4: AllToAll (head-split -> row-split) ----
            cc = nc.gpsimd.collective_compute(
                "AllToAll",
                mybir.AluOpType.bypass,
                ins=[cc_in],
                outs=[cc_out],
                replica_groups=[list(range(NCORES))],
            )
            for wr in cc_writes:
                add_dep_helper(cc.ins, wr.ins, sync=True, reason="cc in ready")

            # ---- phase 5: output projection for own 512 rows ----
            af_sb = singles.tile([128, NCORES, 512], bf16, tag="af")
            cc_out_r = cc_out.rearrange("(i p) q -> p i q", p=128)
            for (lo, hi), eng in (
                ((0, 3), nc.sync),
                ((3, 6), nc.gpsimd),
                ((6, 8), nc.scalar),
            ):
                rd = eng.dma_start(
                    out=af_sb[:, lo:hi, :], in_=cc_out_r[:, lo:hi, :]
                )
                add_dep_helper(rd.ins, cc.ins, sync=True, reason="cc out ready")
            with (
                tc.tile_pool(name="op_ps", bufs=4, space="PSUM") as op,
                tc.tile_pool(name="out_sb", bufs=2) as opool,
            ):
                # keep the PE array warm through the collective window so the
                # output projection doesn't start HAM-throttled: ~13 us of
                # dependency-free dummy matmuls into a scratch psum tile.
                warm = op.tile([128, 512], f32, tag="op", name="warm_ps")
                for i in range(60):
                    nc.tensor.matmul(
                        warm, ident, kT_sb[:, 0:512], start=True, stop=True
                    )
                for r in range(ROWS_PER_CORE // 128):
                    o_sb = opool.tile([128, D], f32, tag="osb")
                    for n in range(D // 512):
                        ps = op.tile([128, 512], f32, tag="op")
                        for i in range(NCORES):
                            nc.tensor.matmul(
                                ps,
                                af_sb[:, i, 128 * r : 128 * (r + 1)],
                                wo_sb[:, i, 512 * n : 512 * (n + 1)],
                                start=(i == 0),
                                stop=False,
                            )
                        nc.tensor.matmul(
                            ps,
                            ones_row,
                            bo_sb[:, 512 * n : 512 * (n + 1)],
                            start=False,
                            stop=True,
                        )
                        nc.scalar.copy(o_sb[:, 512 * n : 512 * (n + 1)], ps)
                    [nc.sync, nc.gpsimd][r % 2].dma_start(
                        out=out[128 * r : 128 * (r + 1), :], in_=o_sb
                    )

    return nc


def _host_prep(x, Wq, bq, Wk, bk, Wv, bv, Wo, bo):
    """Build the 8 per-core input maps."""
    x = np.asarray(x, np.float32)
    xT = np.ascontiguousarray(x.reshape(R, D).T).astype(_BF16)
    woT = np.ascontiguousarray(np.asarray(Wo, np.float32).T).astype(_BF16)
    bo_row = np.asarray(bo, np.float32).reshape(1, D).astype(_BF16)

    # multiplicative causal masks for the 4 diagonal offsets; each 1024-col
    # block m is [mask_m | ones] so a merged strip-pair tile can be masked in
    # one op (second strip fully visible when the first is on the diagonal).
    masks = np.ones((128, 4 * 1024), np.float32)
    r = np.arange(128)[:, None]
    c = np.arange(512)[None, :]
    for m in range(4):
        masks[:, 1024 * m : 1024 * m + 512] = (r + 128 * m <= c).astype(np.float32)
    masks = masks.astype(_BF16)

    in_maps = []
    for core in range(NCORES):
        hs = slice(HDIM * core, HDIM * (core + 1))
        in_maps.append(
            {
                "xT": xT,
                "wqT": np.ascontiguousarray(np.asarray(Wq, np.float32)[hs, :].T).astype(_BF16),
                "wkT": np.ascontiguousarray(np.asarray(Wk, np.float32)[hs, :].T).astype(_BF16),
                "wvT": np.ascontiguousarray(np.asarray(Wv, np.float32)[hs, :].T).astype(_BF16),
                "bq_s": np.asarray(bq, np.float32)[hs].reshape(HDIM, 1).copy(),
                "bk_s": np.asarray(bk, np.float32)[hs].reshape(HDIM, 1).copy(),
                "bv_s": np.asarray(bv, np.float32)[hs].reshape(HDIM, 1).copy(),
                "woT": woT,
                "bo_row": bo_row,
                "masks": masks,
            }
        )
    return in_maps


def _run(in_maps, trace=False):
    from concourse import bass_utils

    if "nc" not in _cache:
        _cache["nc"] = _build()
    nc = _cache["nc"]
    if trace:
        try:
            res = bass_utils.run_bass_kernel_spmd(
                nc, in_maps, core_ids=list(range(NCORES)), trace=True
            )
            return res
        except Exception:
            pass  # NTFF hook unavailable under this axon build
    res = bass_utils.run_bass_kernel_spmd(
        nc, in_maps, core_ids=list(range(NCORES)), trace=False
    )
    return res


def kernel(x, Wq, bq, Wk, bk, Wv, bv, Wo, bo, _trace=False, _want_results=False):
    in_maps = _host_prep(x, Wq, bq, Wk, bk, Wv, bv, Wo, bo)
    res = _run(in_maps, trace=_trace)
    parts = [res.results[c]["out"] for c in range(NCORES)]
    full = np.concatenate(parts, axis=0).reshape(B, T, D).astype(np.float32)
    if _want_results:
        return full, res
    return full
